# revision 11
# baseline (speedup 1.0000x reference)
"""Trainium2 Bass kernel for MiniVandermondeKernel.

Computes kernel[h, l] = sum_p Wc[h, p] * Ac[p]^l  for l in [0, 16384),
with Ac/Wc complex (stored as (...,2) real pairs), |Ac| in [0.9, 0.999).

Strategy
--------
INTERLEAVED L-sharding: core c owns columns l = 8t + c, t in [0, 2048).
Then kernel_c[h, t] = sum_p (Wc*Ac^c)[h,p] * B[p]^t with B = A^8 — a
Vandermonde in B, identical shape on every core (SPMD, no collective).

Within a core, split t into 4 blocks of Lb=512. B^(512j + dt) =
B^(512j) * B^dt, so block j is (Wc * A^(c + 4096j)) @ V0[:, dt] with
V0[p, dt] = B[p]^dt — every block contracts against the SAME stored V0,
with per-block host-precomputed (fp64) weights.

DECAY PRUNING: modes are sorted by |A| descending. A mode of radius r
decays relative to the dominant column scale (~r0^(8t)) as
(r/r0)^(8t); once that ratio is < e^-C (C=18) the mode's contribution
is far below the fp32 noise floor and is dropped:
  - per K-tile k (128 sorted modes), V0 columns are stored only up to
    t_k = C / (8 (|ln r_max(k)| - |ln r0|))  (rounded up to 128, cap 512)
  - block j>0 includes K-tile k only if t_k > 512j, with the matmul N
    clipped to t_k - 512j.
This cuts input DMA ~4x and matmul work ~3x vs the dense version.

Complex matmul via PSUM accumulation with M-packing (H=64 -> M=128):
  pass 1: lhsT = [Wr^T | Wi^T]   rhs = Vr   -> psum  = [Wr@Vr ; Wi@Vr]
  pass 2: lhsT = [-Wi^T | Wr^T]  rhs = Vi   -> psum += [-Wi@Vi ; Wr@Vi]
  => psum = [Kr ; Ki]  (one PSUM bank per block, no vector epilogue)
The pass-2 weights are derived on-device from the pass-1 weights by a
DVE negate + copy (saves shipping them). fp32 data is fed to the PE as
float32r (full-rate fp32 matmul).

Blob layout / pipelining: [W(k<=1) | V0 V1 | W(k>=2) | V2.. ] in DMA
chunks of ~1.25 MB alternating over the two HWDGE rings, so matmuls
start after the first chunk lands and stream behind the DMA. Blocks
1..3 close their PSUM accumulation at small k, so their outputs DMA out
while block 0 is still contracting.
"""
import os
import numpy as np

import concourse.bacc as bacc
import concourse.mybir as mybir
from concourse.tile import TileContext
from concourse.bass_utils import run_bass_kernel_spmd

P = 2048          # d_state
H = 64            # d_input
L = 16384         # kernel_size
NCORES = 8
TCORE = L // NCORES          # 2048 t-columns per core
LB = 512                     # block size (= one PSUM bank of fp32)
NBLK = TCORE // LB           # 4 blocks per core
KT = P // 128                # 16 contraction K-tiles
CUT = 18.0                   # drop modes past (r/r0)^(8t) < e^-CUT
KSPLIT = 2                   # W packs for k < KSPLIT ship in chunk 0
CHUNK_COLS = 950             # ~475 KB fp32 DMA chunk target

_DT = {
    "f32": mybir.dt.float32,
    "f32r": mybir.dt.float32r,
    "bf16": mybir.dt.bfloat16,
}


def _np_dt(dt_name):
    import ml_dtypes
    return np.dtype(ml_dtypes.bfloat16) if dt_name == "bf16" else np.float32


def _ceil64(x):
    return int(min(LB, 64 * np.ceil(max(x, 1) / 64)))


def make_plan(A):
    """Data-dependent pruning plan (hashable)."""
    A = np.asarray(A)
    r = np.hypot(A[:, 0].astype(np.float64), A[:, 1].astype(np.float64))
    rs = np.sort(r)[::-1]
    lr0 = -np.log(rs[0])
    t_raw = [CUT / (8.0 * max(-np.log(rs[128 * k]) - lr0, 1e-9))
             for k in range(KT)]
    budget = tuple(_ceil64(min(t, LB)) for t in t_raw)      # stored V0 cols
    blocks = []
    for j in range(NBLK):
        bl = []
        for k in range(KT):
            rem = t_raw[k] - LB * j
            if k == 0 or rem > 0:
                bl.append((k, _ceil64(min(rem, LB)) if k else LB))
        blocks.append(tuple(bl))
    return budget, tuple(blocks)


def _layout(plan):
    """Blob layout: k-major entry list  [W packs for k | vr_k | vi_k] ...

    Returns (wpairs, off, chunks, total). chunks is a list of
    (start, end, wruns) where wruns is a list of (lo, hi) column ranges
    of W packs inside the chunk.
    """
    budget, blocks = plan
    wpairs = sorted(
        [(j, k) for j, bl in enumerate(blocks) for (k, _) in bl],
        key=lambda jk: (jk[1], jk[0]))
    off = {}
    entries = []             # (start_col, end_col, is_w)
    col = 0
    for k in range(KT):
        for (j, kk) in wpairs:
            if kk == k:
                off[("w", j, k)] = col
                entries.append((col, col + 128, True))
                col += 128
        off[("vr", k)] = col
        entries.append((col, col + budget[k], False))
        col += budget[k]
        off[("vi", k)] = col
        entries.append((col, col + budget[k], False))
        col += budget[k]
    total = col

    chunks = []
    start = 0
    wruns = []
    run = None
    for (a, b, is_w) in entries:
        if is_w:
            if run is not None and run[1] == a:
                run = (run[0], b)
            else:
                if run is not None:
                    wruns.append(run)
                run = (a, b)
        else:
            if run is not None:
                wruns.append(run)
                run = None
        if b - start >= CHUNK_COLS or b == total:
            if run is not None:       # close an open W run at chunk edge
                wruns.append((run[0], b))
                run = (b, b) if b != total else None
                if run is not None and run[0] == run[1]:
                    run = None
            chunks.append((start, b, [r for r in wruns if r[1] > r[0]]))
            start = b
            wruns = []
    return wpairs, off, chunks, total


_compiled = {}


def build_nc(dt_name, plan, loop_iters=1):
    dt = _DT[dt_name]
    budget, blocks = plan
    wpairs, off, chunks, total_cols = _layout(plan)
    nc = bacc.Bacc("TRN2", target_bir_lowering=False, debug=False,
                   num_devices=NCORES)
    blob = nc.dram_tensor("blob", [128, total_cols], dt,
                          kind="ExternalInput").ap()
    out = nc.dram_tensor("out", [128, TCORE], mybir.dt.float32,
                         kind="ExternalOutput").ap()

    def chunk_of(col):
        for i, (a, b, _) in enumerate(chunks):
            if a <= col < b:
                return i
        raise ValueError(col)

    with TileContext(nc) as tc:
        def body():
            with (
                tc.tile_pool(name="csb", bufs=1) as cpool,
                tc.tile_pool(name="wsb", bufs=1) as wpool,
                tc.tile_pool(name="ps", bufs=1, space="PSUM") as pspool,
                tc.tile_pool(name="o", bufs=1) as opool,
            ):
                out_t = opool.tile([128, TCORE], mybir.dt.float32)
                ps = [pspool.tile([128, LB], mybir.dt.float32, tag=f"ps{j}",
                                  name=f"ps{j}") for j in range(NBLK)]
                ct = []
                w2 = {}          # (run_lo) -> (w2 tile, run_lo)
                for i, (a, b, wruns) in enumerate(chunks):
                    t = cpool.tile([128, b - a], dt, tag=f"c{i}",
                                   name=f"ct{i}")
                    eng = nc.sync if i % 2 == 0 else nc.scalar
                    eng.dma_start(out=t[:], in_=blob[:, a:b])
                    ct.append(t)
                    for (lo, hi) in wruns:
                        w2t = wpool.tile([128, hi - lo], dt,
                                         tag=f"w2_{lo}", name=f"w2t{lo}")
                        w1v = t[:, lo - a:hi - a].rearrange(
                            "p (g two m) -> p g two m", two=2, m=64)
                        w2v = w2t.rearrange(
                            "p (g two m) -> p g two m", two=2, m=64)
                        nc.vector.tensor_scalar_mul(
                            w2v[:, :, 0, :], w1v[:, :, 1, :], -1.0)
                        nc.vector.tensor_copy(
                            w2v[:, :, 1, :], w1v[:, :, 0, :])
                        w2[lo] = w2t

                def w_aps(j, k):
                    col = off[("w", j, k)]
                    i = chunk_of(col)
                    a = chunks[i][0]
                    for (lo, hi) in chunks[i][2]:
                        if lo <= col < hi:
                            return (ct[i][:, col - a:col - a + 128],
                                    w2[lo][:, col - lo:col - lo + 128])
                    raise ValueError((j, k))

                def v_ap(kind, k, n):
                    col = off[(kind, k)]
                    i = chunk_of(col)
                    a = chunks[i][0]
                    return ct[i][:, col - a:col - a + n]

                started = set()
                closing = {j: max(k for (k, _) in bl)
                           for j, bl in enumerate(blocks)}
                for k in range(KT):
                    for j, bl in enumerate(blocks):
                        use = dict(bl).get(k)
                        if use is None:
                            continue
                        w1ap, w2ap = w_aps(j, k)
                        first = j not in started
                        started.add(j)
                        last = closing[j] == k
                        nc.tensor.matmul(
                            ps[j][:, 0:use], w1ap, v_ap("vr", k, use),
                            start=first, stop=False)
                        nc.tensor.matmul(
                            ps[j][:, 0:use], w2ap, v_ap("vi", k, use),
                            start=False, stop=last)
                        if last:
                            nc.scalar.copy(out_t[:, j * LB:(j + 1) * LB],
                                           ps[j][:])
                            eng = nc.sync if j % 2 == 0 else nc.scalar
                            eng.dma_start(
                                out=out[:, j * LB:(j + 1) * LB],
                                in_=out_t[:, j * LB:(j + 1) * LB])

        if loop_iters > 1:
            with tc.For_i(0, loop_iters, 1):
                body()
        else:
            body()

    nc.compile()
    return nc


def host_prep(A, W, plan, dt_name):
    """fp64 host-side factorization -> per-core device input blobs."""
    budget, blocks = plan
    wpairs, off, chunks, total_cols = _layout(plan)
    A = np.asarray(A)
    W = np.asarray(W)
    Ac = A[:, 0].astype(np.float64) + 1j * A[:, 1].astype(np.float64)
    Wc = W[..., 0].astype(np.float64) + 1j * W[..., 1].astype(np.float64)
    r = np.abs(Ac)
    order = np.argsort(-r)
    Ac = Ac[order]
    Wc = Wc[:, order]
    logA = np.log(Ac)                        # (P,) complex128
    logB = 8.0 * logA
    npdt = _np_dt(dt_name)

    vparts = {}
    for k in range(KT):
        n = budget[k]
        d = np.arange(n, dtype=np.float64)
        with np.errstate(under="ignore"):
            V = np.exp(logB[128 * k:128 * (k + 1), None] * d[None, :])
        vparts[("vr", k)] = V.real.astype(npdt)
        vparts[("vi", k)] = V.imag.astype(npdt)

    in_maps = []
    with np.errstate(under="ignore"):
        for c in range(NCORES):
            blob = np.zeros((128, total_cols), npdt)
            for (j, k) in wpairs:
                tw = np.exp(logA[128 * k:128 * (k + 1)]
                            * float(c + 8 * LB * j))
                WjT = (Wc[:, 128 * k:128 * (k + 1)] * tw[None, :]).T  # (128,H)
                col = off[("w", j, k)]
                blob[:, col:col + H] = WjT.real.astype(npdt)
                blob[:, col + H:col + 128] = WjT.imag.astype(npdt)
            for k in range(KT):
                for kind in ("vr", "vi"):
                    col = off[(kind, k)]
                    blob[:, col:col + budget[k]] = vparts[(kind, k)]
            in_maps.append({"blob": blob})
    return in_maps


def assemble(results):
    """Per-core (128, 2048) fp32 outputs -> (64, 16384) complex64."""
    K = np.empty((H, L), np.complex64)
    for c in range(NCORES):
        o = results[c]["out"]
        K[:, c::NCORES] = o[0:64] + 1j * o[64:128]
    return K


def _get_nc(dt_name, plan):
    key = (dt_name, plan)
    if key not in _compiled:
        _compiled[key] = build_nc(dt_name, plan)
    return _compiled[key]


def kernel(A, W, kernel_size):
    ks = int(np.asarray(kernel_size))
    assert ks == L, f"kernel_size {ks} != {L} (kernel is shape-specialized)"
    dt_name = os.environ.get("VDM_DT", "f32r")
    plan = make_plan(A)
    nc = _get_nc(dt_name, plan)
    in_maps = host_prep(A, W, plan, dt_name)
    res = run_bass_kernel_spmd(nc, in_maps, core_ids=list(range(NCORES)))
    return assemble(res.results)


# revision 12
# speedup vs baseline: 1.0405x; 1.0405x over previous
"""Trainium2 Bass kernel for MiniVandermondeKernel.

Computes kernel[h, l] = sum_p Wc[h, p] * Ac[p]^l  for l in [0, 16384),
with Ac/Wc complex (stored as (...,2) real pairs), |Ac| in [0.9, 0.999).

Strategy
--------
INTERLEAVED L-sharding: core c owns columns l = 8t + c, t in [0, 2048).
Then kernel_c[h, t] = sum_p (Wc*Ac^c)[h,p] * B[p]^t with B = A^8 — a
Vandermonde in B, identical shape on every core (SPMD, no collective).

Within a core, split t into 4 blocks of Lb=512. B^(512j + dt) =
B^(512j) * B^dt, so block j is (Wc * A^(c + 4096j)) @ V0[:, dt] with
V0[p, dt] = B[p]^dt — every block contracts against the SAME stored V0,
with per-block host-precomputed (fp64) weights.

DECAY PRUNING: modes are sorted by |A| descending. A mode of radius r
decays relative to the dominant column scale (~r0^(8t)) as
(r/r0)^(8t); once that ratio is < e^-C (C=18) the mode's contribution
is far below the fp32 noise floor and is dropped:
  - per K-tile k (128 sorted modes), V0 columns are stored only up to
    t_k = C / (8 (|ln r_max(k)| - |ln r0|))  (rounded up to 128, cap 512)
  - block j>0 includes K-tile k only if t_k > 512j, with the matmul N
    clipped to t_k - 512j.
This cuts input DMA ~4x and matmul work ~3x vs the dense version.

Complex matmul via PSUM accumulation with M-packing (H=64 -> M=128):
  pass 1: lhsT = [Wr^T | Wi^T]   rhs = Vr   -> psum  = [Wr@Vr ; Wi@Vr]
  pass 2: lhsT = [-Wi^T | Wr^T]  rhs = Vi   -> psum += [-Wi@Vi ; Wr@Vi]
  => psum = [Kr ; Ki]  (one PSUM bank per block, no vector epilogue)
The pass-2 weights are derived on-device from the pass-1 weights by a
DVE negate + copy (saves shipping them). fp32 data is fed to the PE as
float32r (full-rate fp32 matmul).

Blob layout / pipelining: [W(k<=1) | V0 V1 | W(k>=2) | V2.. ] in DMA
chunks of ~1.25 MB alternating over the two HWDGE rings, so matmuls
start after the first chunk lands and stream behind the DMA. Blocks
1..3 close their PSUM accumulation at small k, so their outputs DMA out
while block 0 is still contracting.
"""
import os
import numpy as np

import concourse.bacc as bacc
import concourse.mybir as mybir
from concourse.tile import TileContext
from concourse.bass_utils import run_bass_kernel_spmd

P = 2048          # d_state
H = 64            # d_input
L = 16384         # kernel_size
NCORES = 8
TCORE = L // NCORES          # 2048 t-columns per core
LB = 512                     # block size (= one PSUM bank of fp32)
NBLK = TCORE // LB           # 4 blocks per core
KT = P // 128                # 16 contraction K-tiles
CUT = 18.0                   # drop modes past (r/r0)^(8t) < e^-CUT
KSPLIT = 2                   # W packs for k < KSPLIT ship in chunk 0
CHUNK_COLS = 2240            # ~1.1 MB fp32 DMA chunk target

_DT = {
    "f32": mybir.dt.float32,
    "f32r": mybir.dt.float32r,
    "bf16": mybir.dt.bfloat16,
}


def _np_dt(dt_name):
    import ml_dtypes
    return np.dtype(ml_dtypes.bfloat16) if dt_name == "bf16" else np.float32


def _ceil64(x):
    return int(min(LB, 64 * np.ceil(max(x, 1) / 64)))


def make_plan(A):
    """Data-dependent pruning plan (hashable)."""
    A = np.asarray(A)
    r = np.hypot(A[:, 0].astype(np.float64), A[:, 1].astype(np.float64))
    rs = np.sort(r)[::-1]
    lr0 = -np.log(rs[0])
    t_raw = [CUT / (8.0 * max(-np.log(rs[128 * k]) - lr0, 1e-9))
             for k in range(KT)]
    budget = tuple(_ceil64(min(t, LB)) for t in t_raw)      # stored V0 cols
    blocks = []
    for j in range(NBLK):
        bl = []
        for k in range(KT):
            rem = t_raw[k] - LB * j
            if k == 0 or rem > 0:
                bl.append((k, _ceil64(min(rem, LB)) if k else LB))
        blocks.append(tuple(bl))
    return budget, tuple(blocks)


def _layout(plan):
    """Blob layout: k-major entry list  [W packs for k | vr_k | vi_k] ...

    Returns (wpairs, off, chunks, total). chunks is a list of
    (start, end, wruns) where wruns is a list of (lo, hi) column ranges
    of W packs inside the chunk.
    """
    budget, blocks = plan
    wpairs = sorted(
        [(j, k) for j, bl in enumerate(blocks) for (k, _) in bl],
        key=lambda jk: (jk[1], jk[0]))
    off = {}
    entries = []             # (start_col, end_col, is_w)
    col = 0
    for k in range(KT):
        for (j, kk) in wpairs:
            if kk == k:
                off[("w", j, k)] = col
                entries.append((col, col + 128, True))
                col += 128
        off[("vr", k)] = col
        entries.append((col, col + budget[k], False))
        col += budget[k]
        off[("vi", k)] = col
        entries.append((col, col + budget[k], False))
        col += budget[k]
    total = col

    chunks = []
    start = 0
    wruns = []
    run = None
    for (a, b, is_w) in entries:
        if is_w:
            if run is not None and run[1] == a:
                run = (run[0], b)
            else:
                if run is not None:
                    wruns.append(run)
                run = (a, b)
        else:
            if run is not None:
                wruns.append(run)
                run = None
        if b - start >= CHUNK_COLS or b == total:
            if run is not None:       # close an open W run at chunk edge
                wruns.append((run[0], b))
                run = (b, b) if b != total else None
                if run is not None and run[0] == run[1]:
                    run = None
            chunks.append((start, b, [r for r in wruns if r[1] > r[0]]))
            start = b
            wruns = []
    return wpairs, off, chunks, total


_compiled = {}


def build_nc(dt_name, plan, loop_iters=1):
    dt = _DT[dt_name]
    budget, blocks = plan
    wpairs, off, chunks, total_cols = _layout(plan)
    nc = bacc.Bacc("TRN2", target_bir_lowering=False, debug=False,
                   num_devices=NCORES)
    blob = nc.dram_tensor("blob", [128, total_cols], dt,
                          kind="ExternalInput").ap()
    out = nc.dram_tensor("out", [128, TCORE], mybir.dt.float32,
                         kind="ExternalOutput").ap()

    def chunk_of(col):
        for i, (a, b, _) in enumerate(chunks):
            if a <= col < b:
                return i
        raise ValueError(col)

    with TileContext(nc) as tc:
        def body():
            with (
                tc.tile_pool(name="csb", bufs=1) as cpool,
                tc.tile_pool(name="wsb", bufs=1) as wpool,
                tc.tile_pool(name="ps", bufs=1, space="PSUM") as pspool,
                tc.tile_pool(name="o", bufs=1) as opool,
            ):
                out_t = opool.tile([128, TCORE], mybir.dt.float32)
                ps = [pspool.tile([128, LB], mybir.dt.float32, tag=f"ps{j}",
                                  name=f"ps{j}") for j in range(NBLK)]
                ct = []
                w2 = {}          # (run_lo) -> (w2 tile, run_lo)
                for i, (a, b, wruns) in enumerate(chunks):
                    t = cpool.tile([128, b - a], dt, tag=f"c{i}",
                                   name=f"ct{i}")
                    eng = nc.sync if i % 2 == 0 else nc.scalar
                    eng.dma_start(out=t[:], in_=blob[:, a:b])
                    ct.append(t)
                    for (lo, hi) in wruns:
                        w2t = wpool.tile([128, hi - lo], dt,
                                         tag=f"w2_{lo}", name=f"w2t{lo}")
                        w1v = t[:, lo - a:hi - a].rearrange(
                            "p (g two m) -> p g two m", two=2, m=64)
                        w2v = w2t.rearrange(
                            "p (g two m) -> p g two m", two=2, m=64)
                        nc.vector.tensor_scalar_mul(
                            w2v[:, :, 0, :], w1v[:, :, 1, :], -1.0)
                        nc.vector.tensor_copy(
                            w2v[:, :, 1, :], w1v[:, :, 0, :])
                        w2[lo] = w2t

                def w_aps(j, k):
                    col = off[("w", j, k)]
                    i = chunk_of(col)
                    a = chunks[i][0]
                    for (lo, hi) in chunks[i][2]:
                        if lo <= col < hi:
                            return (ct[i][:, col - a:col - a + 128],
                                    w2[lo][:, col - lo:col - lo + 128])
                    raise ValueError((j, k))

                def v_ap(kind, k, n):
                    col = off[(kind, k)]
                    i = chunk_of(col)
                    a = chunks[i][0]
                    return ct[i][:, col - a:col - a + n]

                started = set()
                closing = {j: max(k for (k, _) in bl)
                           for j, bl in enumerate(blocks)}
                for k in range(KT):
                    for j, bl in enumerate(blocks):
                        use = dict(bl).get(k)
                        if use is None:
                            continue
                        w1ap, w2ap = w_aps(j, k)
                        first = j not in started
                        started.add(j)
                        last = closing[j] == k
                        nc.tensor.matmul(
                            ps[j][:, 0:use], w1ap, v_ap("vr", k, use),
                            start=first, stop=False)
                        nc.tensor.matmul(
                            ps[j][:, 0:use], w2ap, v_ap("vi", k, use),
                            start=False, stop=last)
                        if last:
                            nc.scalar.copy(out_t[:, j * LB:(j + 1) * LB],
                                           ps[j][:])
                            eng = nc.sync if j % 2 == 0 else nc.scalar
                            eng.dma_start(
                                out=out[:, j * LB:(j + 1) * LB],
                                in_=out_t[:, j * LB:(j + 1) * LB])

        if loop_iters > 1:
            with tc.For_i(0, loop_iters, 1):
                body()
        else:
            body()

    nc.compile()
    return nc


def host_prep(A, W, plan, dt_name):
    """fp64 host-side factorization -> per-core device input blobs."""
    budget, blocks = plan
    wpairs, off, chunks, total_cols = _layout(plan)
    A = np.asarray(A)
    W = np.asarray(W)
    Ac = A[:, 0].astype(np.float64) + 1j * A[:, 1].astype(np.float64)
    Wc = W[..., 0].astype(np.float64) + 1j * W[..., 1].astype(np.float64)
    r = np.abs(Ac)
    order = np.argsort(-r)
    Ac = Ac[order]
    Wc = Wc[:, order]
    logA = np.log(Ac)                        # (P,) complex128
    logB = 8.0 * logA
    npdt = _np_dt(dt_name)

    vparts = {}
    for k in range(KT):
        n = budget[k]
        d = np.arange(n, dtype=np.float64)
        with np.errstate(under="ignore"):
            V = np.exp(logB[128 * k:128 * (k + 1), None] * d[None, :])
        vparts[("vr", k)] = V.real.astype(npdt)
        vparts[("vi", k)] = V.imag.astype(npdt)

    in_maps = []
    with np.errstate(under="ignore"):
        for c in range(NCORES):
            blob = np.zeros((128, total_cols), npdt)
            for (j, k) in wpairs:
                tw = np.exp(logA[128 * k:128 * (k + 1)]
                            * float(c + 8 * LB * j))
                WjT = (Wc[:, 128 * k:128 * (k + 1)] * tw[None, :]).T  # (128,H)
                col = off[("w", j, k)]
                blob[:, col:col + H] = WjT.real.astype(npdt)
                blob[:, col + H:col + 128] = WjT.imag.astype(npdt)
            for k in range(KT):
                for kind in ("vr", "vi"):
                    col = off[(kind, k)]
                    blob[:, col:col + budget[k]] = vparts[(kind, k)]
            in_maps.append({"blob": blob})
    return in_maps


def assemble(results):
    """Per-core (128, 2048) fp32 outputs -> (64, 16384) complex64."""
    K = np.empty((H, L), np.complex64)
    for c in range(NCORES):
        o = results[c]["out"]
        K[:, c::NCORES] = o[0:64] + 1j * o[64:128]
    return K


def _get_nc(dt_name, plan):
    key = (dt_name, plan)
    if key not in _compiled:
        _compiled[key] = build_nc(dt_name, plan)
    return _compiled[key]


def kernel(A, W, kernel_size):
    ks = int(np.asarray(kernel_size))
    assert ks == L, f"kernel_size {ks} != {L} (kernel is shape-specialized)"
    dt_name = os.environ.get("VDM_DT", "f32r")
    plan = make_plan(A)
    nc = _get_nc(dt_name, plan)
    in_maps = host_prep(A, W, plan, dt_name)
    res = run_bass_kernel_spmd(nc, in_maps, core_ids=list(range(NCORES)))
    return assemble(res.results)


# revision 15
# speedup vs baseline: 1.0892x; 1.0468x over previous
"""Trainium2 Bass kernel for MiniVandermondeKernel.

Computes kernel[h, l] = sum_p Wc[h, p] * Ac[p]^l  for l in [0, 16384),
with Ac/Wc complex (stored as (...,2) real pairs), |Ac| in [0.9, 0.999).

Strategy
--------
INTERLEAVED L-sharding: core c owns columns l = 8t + c, t in [0, 2048).
Then kernel_c[h, t] = sum_p (Wc*Ac^c)[h,p] * B[p]^t with B = A^8 — a
Vandermonde in B, identical shape on every core (SPMD, no collective).

Within a core, split t into 4 blocks of Lb=512. B^(512j + dt) =
B^(512j) * B^dt, so block j is (Wc * A^(c + 4096j)) @ V0[:, dt] with
V0[p, dt] = B[p]^dt — every block contracts against the SAME stored V0,
with per-block host-precomputed (fp64) weights.

DECAY PRUNING: modes are sorted by |A| descending. A mode of radius r
decays relative to the dominant column scale (~r0^(8t)) as
(r/r0)^(8t); once that ratio is < e^-C (C=18) the mode's contribution
is far below the fp32 noise floor and is dropped:
  - per K-tile k (128 sorted modes), V0 columns are stored only up to
    t_k = C / (8 (|ln r_max(k)| - |ln r0|))  (rounded up to 128, cap 512)
  - block j>0 includes K-tile k only if t_k > 512j, with the matmul N
    clipped to t_k - 512j.
This cuts input DMA ~4x and matmul work ~3x vs the dense version.

Complex matmul via PSUM accumulation with M-packing (H=64 -> M=128):
  pass 1: lhsT = [Wr^T | Wi^T]   rhs = Vr   -> psum  = [Wr@Vr ; Wi@Vr]
  pass 2: lhsT = [-Wi^T | Wr^T]  rhs = Vi   -> psum += [-Wi@Vi ; Wr@Vi]
  => psum = [Kr ; Ki]  (one PSUM bank per block, no vector epilogue)
The pass-2 weights are derived on-device from the pass-1 weights by a
DVE negate + copy (saves shipping them). fp32 data is fed to the PE as
float32r (full-rate fp32 matmul).

Blob layout / pipelining: [W(k<=1) | V0 V1 | W(k>=2) | V2.. ] in DMA
chunks of ~1.25 MB alternating over the two HWDGE rings, so matmuls
start after the first chunk lands and stream behind the DMA. Blocks
1..3 close their PSUM accumulation at small k, so their outputs DMA out
while block 0 is still contracting.
"""
import os
import numpy as np

import concourse.bacc as bacc
import concourse.mybir as mybir
from concourse.tile import TileContext
from concourse.bass_utils import run_bass_kernel_spmd

P = 2048          # d_state
H = 64            # d_input
L = 16384         # kernel_size
NCORES = 8
TCORE = L // NCORES          # 2048 t-columns per core
LB = 512                     # block size (= one PSUM bank of fp32)
NBLK = TCORE // LB           # 4 blocks per core
KT = P // 128                # 16 contraction K-tiles
CUT = 18.0                   # drop modes past (r/r0)^(8t) < e^-CUT
KSPLIT = 2                   # W packs for k < KSPLIT ship in chunk 0
CHUNK_COLS = 896             # ~450 KB fp32 DMA chunk target
OUT_GPSIMD = True            # route output DMAs via SWDGE

_DT = {
    "f32": mybir.dt.float32,
    "f32r": mybir.dt.float32r,
    "bf16": mybir.dt.bfloat16,
}


def _np_dt(dt_name):
    import ml_dtypes
    return np.dtype(ml_dtypes.bfloat16) if dt_name == "bf16" else np.float32


def _ceil64(x):
    return int(min(LB, 64 * np.ceil(max(x, 1) / 64)))


def make_plan(A):
    """Data-dependent pruning plan (hashable)."""
    A = np.asarray(A)
    r = np.hypot(A[:, 0].astype(np.float64), A[:, 1].astype(np.float64))
    rs = np.sort(r)[::-1]
    lr0 = -np.log(rs[0])
    t_raw = [CUT / (8.0 * max(-np.log(rs[128 * k]) - lr0, 1e-9))
             for k in range(KT)]
    budget = tuple(_ceil64(min(t, LB)) for t in t_raw)      # stored V0 cols
    blocks = []
    for j in range(NBLK):
        bl = []
        for k in range(KT):
            rem = t_raw[k] - LB * j
            if k == 0 or rem > 0:
                bl.append((k, _ceil64(min(rem, LB)) if k else LB))
        blocks.append(tuple(bl))
    return budget, tuple(blocks)


def _layout(plan):
    """Blob layout: k-major entry list  [W packs for k | vr_k | vi_k] ...

    Returns (wpairs, off, chunks, total). chunks is a list of
    (start, end, wruns) where wruns is a list of (lo, hi) column ranges
    of W packs inside the chunk.
    """
    budget, blocks = plan
    wpairs = sorted(
        [(j, k) for j, bl in enumerate(blocks) for (k, _) in bl],
        key=lambda jk: (jk[1], jk[0]))
    off = {}
    entries = []             # (start_col, end_col, is_w)
    col = 0
    for k in range(KT):
        for (j, kk) in wpairs:
            if kk == k:
                off[("w", j, k)] = col
                entries.append((col, col + 128, True))
                col += 128
        off[("vr", k)] = col
        entries.append((col, col + budget[k], False))
        col += budget[k]
        off[("vi", k)] = col
        entries.append((col, col + budget[k], False))
        col += budget[k]
    total = col

    chunks = []
    start = 0
    wruns = []
    run = None
    for (a, b, is_w) in entries:
        if is_w:
            if run is not None and run[1] == a:
                run = (run[0], b)
            else:
                if run is not None:
                    wruns.append(run)
                run = (a, b)
        else:
            if run is not None:
                wruns.append(run)
                run = None
        if b - start >= CHUNK_COLS or b == total:
            if run is not None:       # close an open W run at chunk edge
                wruns.append((run[0], b))
                run = (b, b) if b != total else None
                if run is not None and run[0] == run[1]:
                    run = None
            chunks.append((start, b, [r for r in wruns if r[1] > r[0]]))
            start = b
            wruns = []
    return wpairs, off, chunks, total


_compiled = {}


def build_nc(dt_name, plan, loop_iters=1, n_body=1):
    dt = _DT[dt_name]
    budget, blocks = plan
    wpairs, off, chunks, total_cols = _layout(plan)
    nc = bacc.Bacc("TRN2", target_bir_lowering=False, debug=False,
                   num_devices=NCORES)
    blob = nc.dram_tensor("blob", [128, total_cols], dt,
                          kind="ExternalInput").ap()
    out = nc.dram_tensor("out", [128, TCORE], mybir.dt.float32,
                         kind="ExternalOutput").ap()

    def chunk_of(col):
        for i, (a, b, _) in enumerate(chunks):
            if a <= col < b:
                return i
        raise ValueError(col)

    with TileContext(nc) as tc:
        def body():
            with (
                tc.tile_pool(name="csb", bufs=1) as cpool,
                tc.tile_pool(name="wsb", bufs=1) as wpool,
                tc.tile_pool(name="ps", bufs=1, space="PSUM") as pspool,
                tc.tile_pool(name="o", bufs=1) as opool,
            ):
                out_t = opool.tile([128, TCORE], mybir.dt.float32)
                ps = [pspool.tile([128, LB], mybir.dt.float32, tag=f"ps{j}",
                                  name=f"ps{j}") for j in range(NBLK)]
                ct = []
                w2 = {}          # (run_lo) -> (w2 tile, run_lo)
                for i, (a, b, wruns) in enumerate(chunks):
                    t = cpool.tile([128, b - a], dt, tag=f"c{i}",
                                   name=f"ct{i}")
                    eng = nc.sync if i % 2 == 0 else nc.scalar
                    eng.dma_start(out=t[:], in_=blob[:, a:b])
                    ct.append(t)
                    for (lo, hi) in wruns:
                        w2t = wpool.tile([128, hi - lo], dt,
                                         tag=f"w2_{lo}", name=f"w2t{lo}")
                        w1v = t[:, lo - a:hi - a].rearrange(
                            "p (g two m) -> p g two m", two=2, m=64)
                        w2v = w2t.rearrange(
                            "p (g two m) -> p g two m", two=2, m=64)
                        nc.vector.tensor_scalar_mul(
                            w2v[:, :, 0, :], w1v[:, :, 1, :], -1.0)
                        nc.vector.tensor_copy(
                            w2v[:, :, 1, :], w1v[:, :, 0, :])
                        w2[lo] = w2t

                def w_aps(j, k):
                    col = off[("w", j, k)]
                    i = chunk_of(col)
                    a = chunks[i][0]
                    for (lo, hi) in chunks[i][2]:
                        if lo <= col < hi:
                            return (ct[i][:, col - a:col - a + 128],
                                    w2[lo][:, col - lo:col - lo + 128])
                    raise ValueError((j, k))

                def v_ap(kind, k, n):
                    col = off[(kind, k)]
                    i = chunk_of(col)
                    a = chunks[i][0]
                    return ct[i][:, col - a:col - a + n]

                started = set()
                closing = {j: max(k for (k, _) in bl)
                           for j, bl in enumerate(blocks)}
                for k in range(KT):
                    for j, bl in enumerate(blocks):
                        use = dict(bl).get(k)
                        if use is None:
                            continue
                        w1ap, w2ap = w_aps(j, k)
                        first = j not in started
                        started.add(j)
                        last = closing[j] == k
                        nc.tensor.matmul(
                            ps[j][:, 0:use], w1ap, v_ap("vr", k, use),
                            start=first, stop=False)
                        nc.tensor.matmul(
                            ps[j][:, 0:use], w2ap, v_ap("vi", k, use),
                            start=False, stop=last)
                        if last:
                            nc.scalar.copy(out_t[:, j * LB:(j + 1) * LB],
                                           ps[j][:])
                            oeng = (nc.gpsimd if OUT_GPSIMD
                                    else (nc.sync if j % 2 == 0
                                          else nc.scalar))
                            oeng.dma_start(
                                out=out[:, j * LB:(j + 1) * LB],
                                in_=out_t[:, j * LB:(j + 1) * LB])

        if loop_iters > 1:
            with tc.For_i(0, loop_iters, 1):
                for _ in range(n_body):
                    body()
        else:
            body()

    nc.compile()
    return nc


def host_prep(A, W, plan, dt_name):
    """fp64 host-side factorization -> per-core device input blobs."""
    budget, blocks = plan
    wpairs, off, chunks, total_cols = _layout(plan)
    A = np.asarray(A)
    W = np.asarray(W)
    Ac = A[:, 0].astype(np.float64) + 1j * A[:, 1].astype(np.float64)
    Wc = W[..., 0].astype(np.float64) + 1j * W[..., 1].astype(np.float64)
    r = np.abs(Ac)
    order = np.argsort(-r)
    Ac = Ac[order]
    Wc = Wc[:, order]
    logA = np.log(Ac)                        # (P,) complex128
    logB = 8.0 * logA
    npdt = _np_dt(dt_name)

    vparts = {}
    for k in range(KT):
        n = budget[k]
        d = np.arange(n, dtype=np.float64)
        with np.errstate(under="ignore"):
            V = np.exp(logB[128 * k:128 * (k + 1), None] * d[None, :])
        vparts[("vr", k)] = V.real.astype(npdt)
        vparts[("vi", k)] = V.imag.astype(npdt)

    in_maps = []
    with np.errstate(under="ignore"):
        for c in range(NCORES):
            blob = np.zeros((128, total_cols), npdt)
            for (j, k) in wpairs:
                tw = np.exp(logA[128 * k:128 * (k + 1)]
                            * float(c + 8 * LB * j))
                WjT = (Wc[:, 128 * k:128 * (k + 1)] * tw[None, :]).T  # (128,H)
                col = off[("w", j, k)]
                blob[:, col:col + H] = WjT.real.astype(npdt)
                blob[:, col + H:col + 128] = WjT.imag.astype(npdt)
            for k in range(KT):
                for kind in ("vr", "vi"):
                    col = off[(kind, k)]
                    blob[:, col:col + budget[k]] = vparts[(kind, k)]
            in_maps.append({"blob": blob})
    return in_maps


def assemble(results):
    """Per-core (128, 2048) fp32 outputs -> (64, 16384) complex64."""
    K = np.empty((H, L), np.complex64)
    for c in range(NCORES):
        o = results[c]["out"]
        K[:, c::NCORES] = o[0:64] + 1j * o[64:128]
    return K


def _get_nc(dt_name, plan):
    key = (dt_name, plan)
    if key not in _compiled:
        _compiled[key] = build_nc(dt_name, plan)
    return _compiled[key]


def kernel(A, W, kernel_size):
    ks = int(np.asarray(kernel_size))
    assert ks == L, f"kernel_size {ks} != {L} (kernel is shape-specialized)"
    dt_name = os.environ.get("VDM_DT", "f32r")
    plan = make_plan(A)
    nc = _get_nc(dt_name, plan)
    in_maps = host_prep(A, W, plan, dt_name)
    res = run_bass_kernel_spmd(nc, in_maps, core_ids=list(range(NCORES)))
    return assemble(res.results)


# revision 16
# speedup vs baseline: 1.2610x; 1.1577x over previous
"""Trainium2 Bass kernel for MiniVandermondeKernel.

Computes kernel[h, l] = sum_p Wc[h, p] * Ac[p]^l  for l in [0, 16384),
with Ac/Wc complex (stored as (...,2) real pairs), |Ac| in [0.9, 0.999).

Strategy
--------
INTERLEAVED L-sharding: core c owns columns l = 8t + c, t in [0, 2048).
Then kernel_c[h, t] = sum_p (Wc*Ac^c)[h,p] * B[p]^t with B = A^8 — a
Vandermonde in B, identical shape on every core (SPMD, no collective).

Within a core, split t into 4 blocks of Lb=512. B^(512j + dt) =
B^(512j) * B^dt, so block j is (Wc * A^(c + 4096j)) @ V0[:, dt] with
V0[p, dt] = B[p]^dt — every block contracts against the SAME stored V0,
with per-block host-precomputed (fp64) weights.

DECAY PRUNING: modes are sorted by |A| descending. A mode of radius r
decays relative to the dominant column scale (~r0^(8t)) as
(r/r0)^(8t); once that ratio is < e^-C (C=18) the mode's contribution
is far below the fp32 noise floor and is dropped:
  - per K-tile k (128 sorted modes), V0 columns are stored only up to
    t_k = C / (8 (|ln r_max(k)| - |ln r0|))  (rounded up to 128, cap 512)
  - block j>0 includes K-tile k only if t_k > 512j, with the matmul N
    clipped to t_k - 512j.
This cuts input DMA ~4x and matmul work ~3x vs the dense version.

Complex matmul via PSUM accumulation with M-packing (H=64 -> M=128):
  pass 1: lhsT = [Wr^T | Wi^T]   rhs = Vr   -> psum  = [Wr@Vr ; Wi@Vr]
  pass 2: lhsT = [-Wi^T | Wr^T]  rhs = Vi   -> psum += [-Wi@Vi ; Wr@Vi]
  => psum = [Kr ; Ki]  (one PSUM bank per block, no vector epilogue)
The pass-2 weights are derived on-device from the pass-1 weights by a
DVE negate + copy (saves shipping them). fp32 data is fed to the PE as
float32r (full-rate fp32 matmul).

Blob layout / pipelining: k-major [W packs(k) | Vr_k | Vi_k] ... in DMA
chunks of ~450 KB alternating over the two HWDGE rings, so matmuls
start after the first chunk lands and stream behind the DMA. Blocks
1..3 close their PSUM accumulation at small k, so their outputs DMA out
(on the gpsimd SWDGE queue, leaving the HWDGE rings to the inputs)
while block 0 is still contracting.
"""
import os
import numpy as np

import concourse.bacc as bacc
import concourse.mybir as mybir
from concourse.tile import TileContext
from concourse.bass_utils import run_bass_kernel_spmd

P = 2048          # d_state
H = 64            # d_input
L = 16384         # kernel_size
NCORES = 8
TCORE = L // NCORES          # 2048 t-columns per core
LB = 512                     # block size (= one PSUM bank of fp32)
NBLK = TCORE // LB           # 4 blocks per core
KT = P // 128                # 16 contraction K-tiles
CUT = 18.0                   # drop modes past (r/r0)^(8t) < e^-CUT
CHUNK_COLS = 896             # ~450 KB fp32 DMA chunk target
OUT_GPSIMD = True            # route output DMAs via SWDGE

_DT = {
    "f32": mybir.dt.float32,
    "f32r": mybir.dt.float32r,
    "bf16": mybir.dt.bfloat16,
}


def _np_dt(dt_name):
    import ml_dtypes
    return np.dtype(ml_dtypes.bfloat16) if dt_name == "bf16" else np.float32


def _ceil64(x):
    return int(min(LB, 64 * np.ceil(max(x, 1) / 64)))


def make_plan(A):
    """Data-dependent pruning plan (hashable)."""
    A = np.asarray(A)
    r = np.hypot(A[:, 0].astype(np.float64), A[:, 1].astype(np.float64))
    rs = np.sort(r)[::-1]
    lr0 = -np.log(rs[0])
    t_raw = [CUT / (8.0 * max(-np.log(rs[128 * k]) - lr0, 1e-9))
             for k in range(KT)]
    budget = tuple(_ceil64(min(t, LB)) for t in t_raw)      # stored V0 cols
    blocks = []
    for j in range(NBLK):
        bl = []
        for k in range(KT):
            rem = t_raw[k] - LB * j
            if k == 0 or rem > 0:
                bl.append((k, _ceil64(min(rem, LB)) if k else LB))
        blocks.append(tuple(bl))
    return budget, tuple(blocks)


def _layout(plan):
    """Blob layout: k-major entry list  [W packs for k | vr_k | vi_k] ...

    Returns (wpairs, off, chunks, total). chunks is a list of
    (start, end, wruns) where wruns is a list of (lo, hi) column ranges
    of W packs inside the chunk.
    """
    budget, blocks = plan
    wpairs = sorted(
        [(j, k) for j, bl in enumerate(blocks) for (k, _) in bl],
        key=lambda jk: (jk[1], jk[0]))
    off = {}
    entries = []             # (start_col, end_col, is_w)
    col = 0
    for k in range(KT):
        for (j, kk) in wpairs:
            if kk == k:
                off[("w", j, k)] = col
                entries.append((col, col + 128, True))
                col += 128
        off[("vr", k)] = col
        entries.append((col, col + budget[k], False))
        col += budget[k]
        off[("vi", k)] = col
        entries.append((col, col + budget[k], False))
        col += budget[k]
    total = col

    chunks = []
    start = 0
    wruns = []
    run = None
    for (a, b, is_w) in entries:
        if is_w:
            if run is not None and run[1] == a:
                run = (run[0], b)
            else:
                if run is not None:
                    wruns.append(run)
                run = (a, b)
        else:
            if run is not None:
                wruns.append(run)
                run = None
        if b - start >= CHUNK_COLS or b == total:
            if run is not None:       # close an open W run at chunk edge
                wruns.append((run[0], b))
                run = (b, b) if b != total else None
                if run is not None and run[0] == run[1]:
                    run = None
            chunks.append((start, b, [r for r in wruns if r[1] > r[0]]))
            start = b
            wruns = []
    return wpairs, off, chunks, total


_compiled = {}


def build_nc(dt_name, plan, loop_iters=1, n_body=1):
    dt = _DT[dt_name]
    budget, blocks = plan
    wpairs, off, chunks, total_cols = _layout(plan)
    nc = bacc.Bacc("TRN2", target_bir_lowering=False, debug=False,
                   num_devices=NCORES)
    blob = nc.dram_tensor("blob", [128, total_cols], dt,
                          kind="ExternalInput").ap()
    out = nc.dram_tensor("out", [128, TCORE], mybir.dt.float32,
                         kind="ExternalOutput").ap()

    def chunk_of(col):
        for i, (a, b, _) in enumerate(chunks):
            if a <= col < b:
                return i
        raise ValueError(col)

    with TileContext(nc) as tc:
        def body():
            with (
                tc.tile_pool(name="csb", bufs=1) as cpool,
                tc.tile_pool(name="wsb", bufs=1) as wpool,
                tc.tile_pool(name="ps", bufs=1, space="PSUM") as pspool,
                tc.tile_pool(name="o", bufs=1) as opool,
            ):
                out_t = opool.tile([128, TCORE], mybir.dt.float32)
                ps = [pspool.tile([128, LB], mybir.dt.float32, tag=f"ps{j}",
                                  name=f"ps{j}") for j in range(NBLK)]
                ct = []
                w2 = {}          # (run_lo) -> (w2 tile, run_lo)
                for i, (a, b, wruns) in enumerate(chunks):
                    t = cpool.tile([128, b - a], dt, tag=f"c{i}",
                                   name=f"ct{i}")
                    eng = nc.sync if i % 2 == 0 else nc.scalar
                    eng.dma_start(out=t[:], in_=blob[:, a:b])
                    ct.append(t)
                    for (lo, hi) in wruns:
                        w2t = wpool.tile([128, hi - lo], dt,
                                         tag=f"w2_{lo}", name=f"w2t{lo}")
                        w1v = t[:, lo - a:hi - a].rearrange(
                            "p (g two m) -> p g two m", two=2, m=64)
                        w2v = w2t.rearrange(
                            "p (g two m) -> p g two m", two=2, m=64)
                        nc.vector.tensor_scalar_mul(
                            w2v[:, :, 0, :], w1v[:, :, 1, :], -1.0)
                        nc.vector.tensor_copy(
                            w2v[:, :, 1, :], w1v[:, :, 0, :])
                        w2[lo] = w2t

                def w_aps(j, k):
                    col = off[("w", j, k)]
                    i = chunk_of(col)
                    a = chunks[i][0]
                    for (lo, hi) in chunks[i][2]:
                        if lo <= col < hi:
                            return (ct[i][:, col - a:col - a + 128],
                                    w2[lo][:, col - lo:col - lo + 128])
                    raise ValueError((j, k))

                def v_ap(kind, k, n):
                    col = off[(kind, k)]
                    i = chunk_of(col)
                    a = chunks[i][0]
                    return ct[i][:, col - a:col - a + n]

                started = set()
                closing = {j: max(k for (k, _) in bl)
                           for j, bl in enumerate(blocks)}
                for k in range(KT):
                    for j, bl in enumerate(blocks):
                        use = dict(bl).get(k)
                        if use is None:
                            continue
                        w1ap, w2ap = w_aps(j, k)
                        first = j not in started
                        started.add(j)
                        last = closing[j] == k
                        nc.tensor.matmul(
                            ps[j][:, 0:use], w1ap, v_ap("vr", k, use),
                            start=first, stop=False)
                        nc.tensor.matmul(
                            ps[j][:, 0:use], w2ap, v_ap("vi", k, use),
                            start=False, stop=last)
                        if last:
                            nc.scalar.copy(out_t[:, j * LB:(j + 1) * LB],
                                           ps[j][:])
                            oeng = (nc.gpsimd if OUT_GPSIMD
                                    else (nc.sync if j % 2 == 0
                                          else nc.scalar))
                            oeng.dma_start(
                                out=out[:, j * LB:(j + 1) * LB],
                                in_=out_t[:, j * LB:(j + 1) * LB])

        if loop_iters > 1:
            with tc.For_i(0, loop_iters, 1):
                for _ in range(n_body):
                    body()
        else:
            body()

    nc.compile()
    return nc


def host_prep(A, W, plan, dt_name):
    """fp64 host-side factorization -> per-core device input blobs."""
    budget, blocks = plan
    wpairs, off, chunks, total_cols = _layout(plan)
    A = np.asarray(A)
    W = np.asarray(W)
    Ac = A[:, 0].astype(np.float64) + 1j * A[:, 1].astype(np.float64)
    Wc = W[..., 0].astype(np.float64) + 1j * W[..., 1].astype(np.float64)
    r = np.abs(Ac)
    order = np.argsort(-r)
    Ac = Ac[order]
    Wc = Wc[:, order]
    logA = np.log(Ac)                        # (P,) complex128
    logB = 8.0 * logA
    npdt = _np_dt(dt_name)

    vparts = {}
    for k in range(KT):
        n = budget[k]
        d = np.arange(n, dtype=np.float64)
        with np.errstate(under="ignore"):
            V = np.exp(logB[128 * k:128 * (k + 1), None] * d[None, :])
        vparts[("vr", k)] = V.real.astype(npdt)
        vparts[("vi", k)] = V.imag.astype(npdt)

    in_maps = []
    with np.errstate(under="ignore"):
        for c in range(NCORES):
            blob = np.zeros((128, total_cols), npdt)
            for (j, k) in wpairs:
                tw = np.exp(logA[128 * k:128 * (k + 1)]
                            * float(c + 8 * LB * j))
                WjT = (Wc[:, 128 * k:128 * (k + 1)] * tw[None, :]).T  # (128,H)
                col = off[("w", j, k)]
                blob[:, col:col + H] = WjT.real.astype(npdt)
                blob[:, col + H:col + 128] = WjT.imag.astype(npdt)
            for k in range(KT):
                for kind in ("vr", "vi"):
                    col = off[(kind, k)]
                    blob[:, col:col + budget[k]] = vparts[(kind, k)]
            in_maps.append({"blob": blob})
    return in_maps


def assemble(results):
    """Per-core (128, 2048) fp32 outputs -> (64, 16384) complex64."""
    K = np.empty((H, L), np.complex64)
    for c in range(NCORES):
        o = results[c]["out"]
        K[:, c::NCORES] = o[0:64] + 1j * o[64:128]
    return K


def _get_nc(dt_name, plan):
    key = (dt_name, plan)
    if key not in _compiled:
        _compiled[key] = build_nc(dt_name, plan)
    return _compiled[key]


def kernel(A, W, kernel_size):
    ks = int(np.asarray(kernel_size))
    assert ks == L, f"kernel_size {ks} != {L} (kernel is shape-specialized)"
    dt_name = os.environ.get("VDM_DT", "f32r")
    plan = make_plan(A)
    nc = _get_nc(dt_name, plan)
    in_maps = host_prep(A, W, plan, dt_name)
    res = run_bass_kernel_spmd(nc, in_maps, core_ids=list(range(NCORES)))
    return assemble(res.results)


# revision 17
# speedup vs baseline: 1.2776x; 1.0131x over previous
"""Trainium2 Bass kernel for MiniVandermondeKernel.

Computes kernel[h, l] = sum_p Wc[h, p] * Ac[p]^l  for l in [0, 16384),
with Ac/Wc complex (stored as (...,2) real pairs), |Ac| in [0.9, 0.999).

Strategy
--------
INTERLEAVED L-sharding: core c owns columns l = 8t + c, t in [0, 2048).
Then kernel_c[h, t] = sum_p (Wc*Ac^c)[h,p] * B[p]^t with B = A^8 — a
Vandermonde in B, identical shape on every core (SPMD, no collective).

Within a core, split t into 4 blocks of Lb=512. B^(512j + dt) =
B^(512j) * B^dt, so block j is (Wc * A^(c + 4096j)) @ V0[:, dt] with
V0[p, dt] = B[p]^dt — every block contracts against the SAME stored V0,
with per-block host-precomputed (fp64) weights.

DECAY PRUNING: modes are sorted by |A| descending. A mode of radius r
decays relative to the dominant column scale (~r0^(8t)) as
(r/r0)^(8t); once that ratio is < e^-C (C=18) the mode's contribution
is far below the fp32 noise floor and is dropped:
  - per K-tile k (128 sorted modes), V0 columns are stored only up to
    t_k = C / (8 (|ln r_max(k)| - |ln r0|))  (rounded up to 128, cap 512)
  - block j>0 includes K-tile k only if t_k > 512j, with the matmul N
    clipped to t_k - 512j.
This cuts input DMA ~4x and matmul work ~3x vs the dense version.

Complex matmul via PSUM accumulation with M-packing (H=64 -> M=128):
  pass 1: lhsT = [Wr^T | Wi^T]   rhs = Vr   -> psum  = [Wr@Vr ; Wi@Vr]
  pass 2: lhsT = [-Wi^T | Wr^T]  rhs = Vi   -> psum += [-Wi@Vi ; Wr@Vi]
  => psum = [Kr ; Ki]  (one PSUM bank per block, no vector epilogue)
The pass-2 weights are derived on-device from the pass-1 weights by a
DVE negate + copy (saves shipping them). fp32 data is fed to the PE as
float32r (full-rate fp32 matmul).

Blob layout / pipelining: k-major [W packs(k) | Vr_k | Vi_k] ... in DMA
chunks of ~450 KB alternating over the two HWDGE rings, so matmuls
start after the first chunk lands and stream behind the DMA. Blocks
1..3 close their PSUM accumulation at small k, so their outputs DMA out
(on the gpsimd SWDGE queue, leaving the HWDGE rings to the inputs)
while block 0 is still contracting.
"""
import os
import numpy as np

import concourse.bacc as bacc
import concourse.mybir as mybir
from concourse.tile import TileContext
from concourse.bass_utils import run_bass_kernel_spmd

P = 2048          # d_state
H = 64            # d_input
L = 16384         # kernel_size
NCORES = 8
TCORE = L // NCORES          # 2048 t-columns per core
LB = 512                     # block size (= one PSUM bank of fp32)
NBLK = TCORE // LB           # 4 blocks per core
KT = P // 128                # 16 contraction K-tiles
CUT = 18.0                   # drop modes past (r/r0)^(8t) < e^-CUT
CHUNK_COLS = 896             # ~450 KB fp32 DMA chunk target
OUT_GPSIMD = True            # route output DMAs via SWDGE

_DT = {
    "f32": mybir.dt.float32,
    "f32r": mybir.dt.float32r,
    "bf16": mybir.dt.bfloat16,
}


def _np_dt(dt_name):
    import ml_dtypes
    return np.dtype(ml_dtypes.bfloat16) if dt_name == "bf16" else np.float32


def _ceil64(x):
    return int(min(LB, 64 * np.ceil(max(x, 1) / 64)))


def make_plan(A):
    """Data-dependent pruning plan (hashable)."""
    A = np.asarray(A)
    r = np.hypot(A[:, 0].astype(np.float64), A[:, 1].astype(np.float64))
    rs = np.sort(r)[::-1]
    lr0 = -np.log(rs[0])
    t_raw = [CUT / (8.0 * max(-np.log(rs[128 * k]) - lr0, 1e-9))
             for k in range(KT)]
    budget = tuple(_ceil64(min(t, LB)) for t in t_raw)      # stored V0 cols
    blocks = []
    for j in range(NBLK):
        bl = []
        for k in range(KT):
            rem = t_raw[k] - LB * j
            if k == 0 or rem > 0:
                bl.append((k, _ceil64(min(rem, LB)) if k else LB))
        blocks.append(tuple(bl))
    return budget, tuple(blocks)


def _layout(plan):
    """Blob layout: k-major entry list  [W packs for k | vr_k | vi_k] ...

    Returns (wpairs, off, chunks, total). chunks is a list of
    (start, end, wruns) where wruns is a list of (lo, hi) column ranges
    of W packs inside the chunk.
    """
    budget, blocks = plan
    wpairs = sorted(
        [(j, k) for j, bl in enumerate(blocks) for (k, _) in bl],
        key=lambda jk: (jk[1], jk[0]))
    off = {}
    entries = []             # (start_col, end_col, is_w)
    col = 0
    for k in range(KT):
        for (j, kk) in wpairs:
            if kk == k:
                off[("w", j, k)] = col
                entries.append((col, col + 128, True))
                col += 128
        off[("vr", k)] = col
        entries.append((col, col + budget[k], False))
        col += budget[k]
        off[("vi", k)] = col
        entries.append((col, col + budget[k], False))
        col += budget[k]
    total = col

    chunks = []
    start = 0
    wruns = []
    run = None
    for (a, b, is_w) in entries:
        if is_w:
            if run is not None and run[1] == a:
                run = (run[0], b)
            else:
                if run is not None:
                    wruns.append(run)
                run = (a, b)
        else:
            if run is not None:
                wruns.append(run)
                run = None
        if b - start >= CHUNK_COLS or b == total:
            if run is not None:       # close an open W run at chunk edge
                wruns.append((run[0], b))
                run = (b, b) if b != total else None
                if run is not None and run[0] == run[1]:
                    run = None
            chunks.append((start, b, [r for r in wruns if r[1] > r[0]]))
            start = b
            wruns = []
    return wpairs, off, chunks, total


_compiled = {}


def build_nc(dt_name, plan, loop_iters=1, n_body=1):
    dt = _DT[dt_name]
    budget, blocks = plan
    wpairs, off, chunks, total_cols = _layout(plan)
    nc = bacc.Bacc("TRN2", target_bir_lowering=False, debug=False,
                   num_devices=NCORES)
    blob = nc.dram_tensor("blob", [128, total_cols], dt,
                          kind="ExternalInput").ap()
    out = nc.dram_tensor("out", [128, TCORE], mybir.dt.float32,
                         kind="ExternalOutput").ap()

    def chunk_of(col):
        for i, (a, b, _) in enumerate(chunks):
            if a <= col < b:
                return i
        raise ValueError(col)

    with TileContext(nc) as tc:
        def body():
            with (
                tc.tile_pool(name="csb", bufs=1) as cpool,
                tc.tile_pool(name="wsb", bufs=1) as wpool,
                tc.tile_pool(name="ps", bufs=1, space="PSUM") as pspool,
                tc.tile_pool(name="o", bufs=1) as opool,
            ):
                out_t = opool.tile([128, TCORE], mybir.dt.float32)
                ps = [pspool.tile([128, LB], mybir.dt.float32, tag=f"ps{j}",
                                  name=f"ps{j}") for j in range(NBLK)]
                ct = []
                w2 = {}          # (run_lo) -> (w2 tile, run_lo)
                for i, (a, b, wruns) in enumerate(chunks):
                    t = cpool.tile([128, b - a], dt, tag=f"c{i}",
                                   name=f"ct{i}")
                    eng = nc.sync if i % 2 == 0 else nc.scalar
                    eng.dma_start(out=t[:], in_=blob[:, a:b])
                    ct.append(t)
                    for (lo, hi) in wruns:
                        w2t = wpool.tile([128, hi - lo], dt,
                                         tag=f"w2_{lo}", name=f"w2t{lo}")
                        w1v = t[:, lo - a:hi - a].rearrange(
                            "p (g two m) -> p g two m", two=2, m=64)
                        w2v = w2t.rearrange(
                            "p (g two m) -> p g two m", two=2, m=64)
                        nc.vector.tensor_scalar_mul(
                            w2v[:, :, 0, :], w1v[:, :, 1, :], -1.0)
                        nc.vector.tensor_copy(
                            w2v[:, :, 1, :], w1v[:, :, 0, :])
                        w2[lo] = w2t

                def w_aps(j, k):
                    col = off[("w", j, k)]
                    i = chunk_of(col)
                    a = chunks[i][0]
                    for (lo, hi) in chunks[i][2]:
                        if lo <= col < hi:
                            return (ct[i][:, col - a:col - a + 128],
                                    w2[lo][:, col - lo:col - lo + 128])
                    raise ValueError((j, k))

                def v_ap(kind, k, n):
                    col = off[(kind, k)]
                    i = chunk_of(col)
                    a = chunks[i][0]
                    return ct[i][:, col - a:col - a + n]

                started = set()
                closing = {j: max(k for (k, _) in bl)
                           for j, bl in enumerate(blocks)}
                for k in range(KT):
                    for j, bl in enumerate(blocks):
                        use = dict(bl).get(k)
                        if use is None:
                            continue
                        w1ap, w2ap = w_aps(j, k)
                        first = j not in started
                        started.add(j)
                        last = closing[j] == k
                        nc.tensor.matmul(
                            ps[j][:, 0:use], w1ap, v_ap("vr", k, use),
                            start=first, stop=False)
                        nc.tensor.matmul(
                            ps[j][:, 0:use], w2ap, v_ap("vi", k, use),
                            start=False, stop=last)
                        if last:
                            nc.vector.tensor_copy(
                                out_t[:, j * LB:(j + 1) * LB], ps[j][:])
                            oeng = (nc.gpsimd if OUT_GPSIMD
                                    else (nc.sync if j % 2 == 0
                                          else nc.scalar))
                            oeng.dma_start(
                                out=out[:, j * LB:(j + 1) * LB],
                                in_=out_t[:, j * LB:(j + 1) * LB])

        if loop_iters > 1:
            with tc.For_i(0, loop_iters, 1):
                for _ in range(n_body):
                    body()
        else:
            body()

    nc.compile()
    return nc


def host_prep(A, W, plan, dt_name):
    """fp64 host-side factorization -> per-core device input blobs."""
    budget, blocks = plan
    wpairs, off, chunks, total_cols = _layout(plan)
    A = np.asarray(A)
    W = np.asarray(W)
    Ac = A[:, 0].astype(np.float64) + 1j * A[:, 1].astype(np.float64)
    Wc = W[..., 0].astype(np.float64) + 1j * W[..., 1].astype(np.float64)
    r = np.abs(Ac)
    order = np.argsort(-r)
    Ac = Ac[order]
    Wc = Wc[:, order]
    logA = np.log(Ac)                        # (P,) complex128
    logB = 8.0 * logA
    npdt = _np_dt(dt_name)

    vparts = {}
    for k in range(KT):
        n = budget[k]
        d = np.arange(n, dtype=np.float64)
        with np.errstate(under="ignore"):
            V = np.exp(logB[128 * k:128 * (k + 1), None] * d[None, :])
        vparts[("vr", k)] = V.real.astype(npdt)
        vparts[("vi", k)] = V.imag.astype(npdt)

    in_maps = []
    with np.errstate(under="ignore"):
        for c in range(NCORES):
            blob = np.zeros((128, total_cols), npdt)
            for (j, k) in wpairs:
                tw = np.exp(logA[128 * k:128 * (k + 1)]
                            * float(c + 8 * LB * j))
                WjT = (Wc[:, 128 * k:128 * (k + 1)] * tw[None, :]).T  # (128,H)
                col = off[("w", j, k)]
                blob[:, col:col + H] = WjT.real.astype(npdt)
                blob[:, col + H:col + 128] = WjT.imag.astype(npdt)
            for k in range(KT):
                for kind in ("vr", "vi"):
                    col = off[(kind, k)]
                    blob[:, col:col + budget[k]] = vparts[(kind, k)]
            in_maps.append({"blob": blob})
    return in_maps


def assemble(results):
    """Per-core (128, 2048) fp32 outputs -> (64, 16384) complex64."""
    K = np.empty((H, L), np.complex64)
    for c in range(NCORES):
        o = results[c]["out"]
        K[:, c::NCORES] = o[0:64] + 1j * o[64:128]
    return K


def _get_nc(dt_name, plan):
    key = (dt_name, plan)
    if key not in _compiled:
        _compiled[key] = build_nc(dt_name, plan)
    return _compiled[key]


def kernel(A, W, kernel_size):
    ks = int(np.asarray(kernel_size))
    assert ks == L, f"kernel_size {ks} != {L} (kernel is shape-specialized)"
    dt_name = os.environ.get("VDM_DT", "f32r")
    plan = make_plan(A)
    nc = _get_nc(dt_name, plan)
    in_maps = host_prep(A, W, plan, dt_name)
    res = run_bass_kernel_spmd(nc, in_maps, core_ids=list(range(NCORES)))
    return assemble(res.results)
